# revision 1
# baseline (speedup 1.0000x reference)
"""Multi-head causal attention on 8 Trainium2 NeuronCores.

Sharding: core c handles batch b=c//4, head group g=c%4 (4 heads of 16).
Per-core Bass kernel computes QKV projection, causal flash-style attention
(transposed-scores layout), and the out-projection partial; the host sums
the 4 per-batch partials (the out_proj all-reduce) and adds the bias.

Layout notes (per core, S=2048 tokens, D=1024, 4 heads x dh=64):
  - xT [D, S] bf16 arrives pre-transposed from host (d_in on partitions).
  - qT/kT [128, pair, S]: partitions = head-dim; pair p holds heads 2p
    (partitions 0:64) and 2p+1 (64:128).
  - scoresT tile [128 k-tokens, 512 q-tokens] = kT_slice.T @ qT_slice with
    K=dh=64 contraction: the two heads of a pair run as concurrent
    row-tiled matmuls (tile_position (0,0) and (64,0)).
  - exp: one ACT instruction per 4 score tiles (A_i,B_i,A_i+1,B_i+1 merged
    in a [128,2048] 4-bank PSUM tile) to amortize the ~352-cycle overhead.
  - ctx^T [dh, q] accumulates over k-chunks as col-tiled dual matmuls
    (head A -> psum partitions 0:64, head B -> 64:128); denominator rows
    come from parallel col-tiled ones-matmuls, replicated 64x so the
    normalization multiply is partition-aligned.
  - out^T partial [D, S] = wo.T @ ctxT, accumulated over the 2 pairs.
"""

import sys

sys.path.insert(0, "/opt/trn_rl_repo")

import numpy as np
import ml_dtypes

import concourse.bass as bass
import concourse.tile as tile
from concourse import bacc, mybir
from concourse import bass_utils

BF16 = ml_dtypes.bfloat16
F32 = mybir.dt.float32
BF = mybir.dt.bfloat16

N_CORES = 8
S = 2048          # tokens
D = 1024          # model dim
DHC = 256         # head dims per core (4 heads x 64)
DH = 64
NQ = 4            # q chunks of 512
NK = 16           # k chunks of 128
NO = 8            # d_in / d_out chunks of 128

_NC_CACHE = None


def _build_core_kernel():
    nc = bacc.Bacc("TRN2", target_bir_lowering=False, debug=False,
                   num_devices=N_CORES)
    xT = nc.dram_tensor("xT", [D, S], BF, kind="ExternalInput").ap()
    w_all = nc.dram_tensor("w_all", [D, 3 * DHC], BF, kind="ExternalInput").ap()
    wo = nc.dram_tensor("wo", [DHC, D], BF, kind="ExternalInput").ap()
    masks = nc.dram_tensor("masks", [128, 4 * 512], BF, kind="ExternalInput").ap()
    outT = nc.dram_tensor("outT", [D, S], F32, kind="ExternalOutput").ap()

    with tile.TileContext(nc) as tc:
        _emit(tc, xT, w_all, wo, masks, outT)
    nc.compile()
    return nc


def _emit(tc, xT, w_all, wo, masks, outT):
    nc = tc.nc
    EXPF = mybir.ActivationFunctionType.Exp

    from contextlib import ExitStack
    ctx = ExitStack()
    const = ctx.enter_context(tc.tile_pool(name="const", bufs=1))
    work = ctx.enter_context(tc.tile_pool(name="work", bufs=4))
    outp = ctx.enter_context(tc.tile_pool(name="outp", bufs=2))
    ps_mm = ctx.enter_context(tc.tile_pool(name="ps_mm", bufs=2, space="PSUM"))
    ps_s = ctx.enter_context(tc.tile_pool(name="ps_s", bufs=2, space="PSUM"))
    ps_c = ctx.enter_context(tc.tile_pool(name="ps_c", bufs=2, space="PSUM"))
    ps_d = ps_mm  # denominator psum is transient now; share the mm slots

    # ---- persistent SBUF tensors ----
    xt = const.tile([128, NO, S], BF, tag="xt")          # x^T, d_in chunks
    wa = const.tile([128, NO, 3 * DHC], BF, tag="wa")    # [Wq|Wk|Wv] slices
    wos = const.tile([128, 2, D], BF, tag="wos")         # Wo row chunks
    msk = const.tile([128, 4, 512], BF, tag="msk")       # causal staircases
    qt = const.tile([128, 2, S], BF, tag="qt")           # q^T per pair
    # k^T zero-padded per head so score matmuls are full-array (K=128):
    # ktpA[:, p] = [kT_head2p | 0], ktpB[:, p] = [0 | kT_head2p+1]
    ktpA = const.tile([128, 2, S], BF, tag="ktpA")
    ktpB = const.tile([128, 2, S], BF, tag="ktpB")
    # v zero-padded per head parity: even head slot = [v|0], odd = [0|v],
    # so ctx matmuls are full-array (M=128) and the two heads' outputs
    # accumulate additively in one PSUM bank.
    vsb = const.tile([128, NK, 4 * 128], BF, tag="vsb")
    ctxT = const.tile([128, 2, S], BF, tag="ctxT")       # ctx^T (normalized
    #                                                      in the post-pass)
    den_all = const.tile([128, 8, 512], F32, tag="den")  # per-chunk denoms
    ones = const.tile([128, DH], BF, tag="ones")

    def chunk_index(p, j):
        return 4 * p + j

    nc.sync.dma_start(wa[:], w_all.rearrange("(o p) f -> p o f", p=128))
    nc.sync.dma_start(msk[:], masks.rearrange("p (d f) -> p d f", f=512))
    nc.sync.dma_start(wos[:], wo.rearrange("(c p) f -> p c f", p=128))
    xTo = xT.rearrange("(o p) s -> o p s", p=128)
    for o in range(NO):  # per-chunk DMAs so matmuls start with chunk 0
        nc.sync.dma_start(xt[:, o, :], xTo[o])
    nc.vector.memset(ones[:], 1.0)
    # contiguous full-tile zero fills; data copies overwrite the live parts
    nc.gpsimd.memset(ktpA[:], 0.0)
    nc.gpsimd.memset(ktpB[:], 0.0)
    nc.gpsimd.memset(vsb[:], 0.0)

    # ---- QKV projections ----
    def emit_qk(m, streaming=False):
        # qkvT chunk m: [128 dims, S] = w_all[:, m-slice].T @ x^T
        # streaming=True: o-outer loop so work starts as x^T chunks land.
        pp = m % 2
        if streaming:
            pq0 = ps_s.tile([128, 1024], F32, tag="ps")
            pq1 = ps_s.tile([128, 1024], F32, tag="ps")
            pqs = [pq0, pq1]
            for o in range(NO):
                for n in range(NQ):
                    nc.tensor.matmul(
                        pqs[n // 2][:, 512 * (n % 2):512 * (n % 2) + 512],
                        lhsT=wa[:, o, 128 * m:128 * m + 128],
                        rhs=xt[:, o, 512 * n:512 * n + 512],
                        start=(o == 0), stop=(o == NO - 1),
                        skip_group_check=True)
        for n in range(NQ):
            n_sl = slice(512 * n, 512 * n + 512)
            if streaming:
                pq = pqs[n // 2][:, 512 * (n % 2):512 * (n % 2) + 512]
            else:
                pq = ps_mm.tile([128, 512], F32, tag="mm")
                for o in range(NO):
                    nc.tensor.matmul(
                        pq[:], lhsT=wa[:, o, 128 * m:128 * m + 128],
                        rhs=xt[:, o, n_sl],
                        start=(o == 0), stop=(o == NO - 1))
            if m < 2:
                nc.vector.tensor_copy(qt[:, pp, n_sl], pq[:])
            else:
                nc.vector.tensor_copy(ktpA[0:64, pp, n_sl], pq[0:64, :])
                nc.vector.tensor_copy(ktpB[64:128, pp, n_sl], pq[64:128, :])

    def emit_v():
        # v [tokens, 4*dh] = x @ Wv  (x^T chunks are the stationary side)
        for t in range(NK):
            pv = ps_mm.tile([128, 512], F32, tag="mm")
            for o in range(NO):
                nc.tensor.matmul(
                    pv[:, :DHC], lhsT=xt[:, o, 128 * t:128 * t + 128],
                    rhs=wa[:, o, 2 * DHC:3 * DHC],
                    start=(o == 0), stop=(o == NO - 1))
            pv4 = pv[:, :DHC].rearrange("p (h c) -> p h c", c=DH)
            dst4 = vsb.rearrange("p t (h c) -> p t h c", c=128)
            # even head slots hold [v|0], odd hold [0|v]
            nc.vector.tensor_copy(dst4[:, t, 0::2, 0:64], pv4[:, 0::2, :])
            nc.vector.tensor_copy(dst4[:, t, 1::2, 64:128], pv4[:, 1::2, :])

    # ---- attention for one (pair, q-chunk) ----
    def emit_attn_chunk(p, j):
        h0 = 2 * p
        n_i = 4 * j + 4
        q_sl = slice(512 * j, 512 * j + 512)
        pc = ps_c.tile([128, 512], F32, tag="pc")
        rs = work.tile([128, 1024], BF, tag="rs")  # per-head exp row-sums
        for i in range(n_i):
            k_sl = slice(128 * i, 128 * i + 128)
            d = i - 4 * j
            # diagonal tiles: k-chunk i only reaches q >= 128*d in this
            # q-window; restrict all work to the valid column range.
            q0 = 128 * d if d > 0 else 0
            qv_sl = slice(512 * j + q0, 512 * j + 512)
            pss = ps_s.tile([128, 1024], F32, tag="ps")
            nc.tensor.matmul(pss[:, q0:512],
                             lhsT=ktpA[:, p, k_sl], rhs=qt[:, p, qv_sl],
                             start=True, stop=True)
            nc.tensor.matmul(pss[:, 512 + q0:1024],
                             lhsT=ktpB[:, p, k_sl], rhs=qt[:, p, qv_sl],
                             start=True, stop=True)
            eT = work.tile([128, 1024], BF, tag="exp")
            if q0:
                ev = eT.rearrange("p (g f) -> p g f", g=2)[:, :, q0:512]
                pv = pss.rearrange("p (g f) -> p g f", g=2)[:, :, q0:512]
                nc.scalar.activation(ev, pv, EXPF, scale=0.125)
            else:
                nc.scalar.activation(eT[:], pss[:], EXPF, scale=0.125)
            if d >= 0:  # triangular 128x128 mask block on the diagonal
                for h in (0, 1):
                    tri = slice(512 * h + q0, 512 * h + q0 + 128)
                    nc.vector.tensor_mul(eT[:, tri], eT[:, tri],
                                         msk[:, 0, :128])
            # row-sum accumulation for the softmax denominators (bf16)
            if q0:
                rv = rs.rearrange("p (g f) -> p g f", g=2)[:, :, q0:512]
                if i == 0:
                    nc.vector.tensor_copy(rv, ev)
                else:
                    nc.vector.tensor_add(rv, rv, ev)
            elif i == 0:
                nc.vector.tensor_copy(rs[:], eT[:])
            else:
                nc.vector.tensor_add(rs[:], rs[:], eT[:])
            for h in (0, 1):  # head within pair; full-array M=128 matmuls
                sl = slice(512 * h + q0, 512 * h + 512)
                hl = h0 + h
                nc.tensor.matmul(
                    pc[:, q0:512],
                    lhsT=vsb[:, i, 128 * hl:128 * hl + 128],
                    rhs=eT[:, sl],
                    start=(i == 0 and h == 0), stop=(i == n_i - 1 and h == 1),
                    skip_group_check=True)
        # denominators: one col-tiled ones-matmul pair on the summed rows
        pd = ps_d.tile([128, 512], F32, tag="mm")
        nc.tensor.matmul(pd[0:64, :], lhsT=ones[:], rhs=rs[:, 0:512],
                         start=True, stop=True)
        nc.tensor.matmul(pd[64:128, :], lhsT=ones[:], rhs=rs[:, 512:1024],
                         start=True, stop=True)
        # stage unnormalized ctx and denominators; 1/d applied post-attention
        ci = chunk_index(p, j)
        nc.vector.tensor_copy(den_all[:, ci, :], pd[:])
        nc.vector.tensor_copy(ctxT[:, p, q_sl], pc[:])

    # ---- softmax normalization for one chunk: ctxT *= exp(-ln(den)) ----
    rec_all = const.tile([128, 8, 512], F32, tag="rec")

    def emit_normalize(p, j):
        ci = chunk_index(p, j)
        q_sl = slice(512 * j, 512 * j + 512)
        nc.scalar.activation(rec_all[:, ci, :], den_all[:, ci, :],
                             mybir.ActivationFunctionType.Ln)
        nc.scalar.activation(rec_all[:, ci, :], rec_all[:, ci, :], EXPF,
                             scale=-1.0)
        nc.vector.tensor_mul(ctxT[:, p, q_sl], ctxT[:, p, q_sl],
                             rec_all[:, ci, :])

    # ---- out projection for one token block: outT[:, n] += wo.T @ ctxT ----
    outT_m = outT.rearrange("(mm p) s -> mm p s", p=128)

    def emit_outproj_n(n):
        n_sl = slice(512 * n, 512 * n + 512)
        for m in range(NO):
            po = ps_mm.tile([128, 512], F32, tag="mm")
            for p in (0, 1):
                nc.tensor.matmul(
                    po[:], lhsT=wos[:, p, 128 * m:128 * m + 128],
                    rhs=ctxT[:, p, n_sl],
                    start=(p == 0), stop=(p == 1))
            osb = outp.tile([128, 512], F32, tag="osb")
            if m % 2 == 0:
                nc.scalar.copy(osb[:], po[:])
            else:
                nc.vector.tensor_copy(osb[:], po[:])
            nc.sync.dma_start(outT_m[m, :, n_sl], osb[:])

    emit_qk(0, streaming=True)
    emit_qk(2, streaming=True)
    emit_v()
    emit_attn_chunk(0, 3)
    emit_qk(1)
    emit_qk(3)
    emit_normalize(0, 3)
    for j in (3, 2, 1, 0):
        if j != 3:
            emit_attn_chunk(0, j)
            emit_normalize(0, j)
        emit_attn_chunk(1, j)
        emit_normalize(1, j)
        emit_outproj_n(j)
    ctx.close()


def _get_nc():
    global _NC_CACHE
    if _NC_CACHE is None:
        _NC_CACHE = _build_core_kernel()
    return _NC_CACHE


def _build_masks():
    p = np.arange(128)[:, None]
    f = np.arange(512)[None, :]
    blocks = [(128 * d + p <= f).astype(BF16) for d in range(4)]
    return np.concatenate(blocks, axis=1)


def _shard_inputs(x, Wq, Wk, Wv, Wo):
    xb = x.astype(BF16)
    masks = _build_masks()
    in_maps = []
    for c in range(N_CORES):
        b, g = divmod(c, 4)
        cols = slice(DHC * g, DHC * g + DHC)
        w_all = np.ascontiguousarray(np.concatenate(
            [Wq[:, cols], Wk[:, cols], Wv[:, cols]], axis=1).astype(BF16))
        wo_s = np.ascontiguousarray(Wo[cols, :].astype(BF16))
        xT = np.ascontiguousarray(xb[b].T)
        in_maps.append({"xT": xT, "w_all": w_all, "wo": wo_s, "masks": masks})
    return in_maps


def _unshard(results, bo):
    out = np.empty((2, S, D), np.float32)
    for b in range(2):
        acc = results[4 * b]["outT"].copy()
        for g in range(1, 4):
            acc += results[4 * b + g]["outT"]
        out[b] = acc.T + bo.astype(np.float32)
    return out


def run(x, Wq, Wk, Wv, Wo, bo, trace=False, **spmd_kwargs):
    nc = _get_nc()
    in_maps = _shard_inputs(x, Wq, Wk, Wv, Wo)
    res = bass_utils.run_bass_kernel_spmd(
        nc, in_maps, core_ids=list(range(N_CORES)), trace=trace,
        **spmd_kwargs)
    return _unshard(res.results, bo), res


def kernel(x, Wq, Wk, Wv, Wo, bo):
    out, _ = run(np.asarray(x), np.asarray(Wq), np.asarray(Wk),
                 np.asarray(Wv), np.asarray(Wo), np.asarray(bo))
    return out



# revision 8
# speedup vs baseline: 1.1411x; 1.1411x over previous
"""Multi-head causal attention on 8 Trainium2 NeuronCores.

Sharding: core c handles batch b=c//4, head group g=c%4 (4 heads of 16).
Per-core Bass kernel computes QKV projection, causal attention in a
transposed-scores layout, and the out-projection partial; the host sums
the 4 per-batch bf16 partials (the out_proj all-reduce) in fp32 + bias.

v2 layout notes (per core, S=2048 tokens, D=1024, 4 heads x dh=64):
  - qt/kt [128, pair, S] bf16: partitions 0:64 = even head dh, 64:128 =
    odd head dh. No zero padding.
  - scores: the two heads of a pair run as CONCURRENT row-tiled K=64
    matmuls (tile_position (0,0) and (64,0)) into two PSUM banks of one
    [128, 1024] tile -> ~2x score throughput vs serial K=128.
  - exp: one ACT per k-chunk over both heads' banks ([128, 2, 512-q0]).
  - ctx: per head one matmul accumulating over k-chunks. A ones-column
    is folded into the V stationary operand so the softmax DENOMINATOR
    accumulates in the same PSUM bank for free (no DVE row-sum chain):
      even head: lhsT = [v(64) | 1] (M=65)  -> ctx at rows 0:64, den row 64
      odd head:  lhsT = [0(32)|1|0(31)|v(64)] (M=128) -> den row 32,
                 ctx at rows 64:128 (lane-aligned with ctxT's B half).
  - denominators: DVE reciprocal_approx_fast on the den rows, then K=1
    ones-matmuls (bf16 x f32r) replicate 1/den across 64 partitions;
    ctxT (staged raw in bf16) is normalized in place by two DVE muls.
  - out^T partial [D, S] bf16 = wo.T @ ctxT, accumulated over the 2
    pairs; host sums partials in fp32.
"""

import sys

sys.path.insert(0, "/opt/trn_rl_repo")

import numpy as np
import ml_dtypes

import concourse.bass as bass
import concourse.tile as tile
from concourse import bacc, mybir
from concourse import bass_utils

BF16 = ml_dtypes.bfloat16
F32 = mybir.dt.float32
F32R = mybir.dt.float32r
BF = mybir.dt.bfloat16

N_CORES = 8
S = 2048          # tokens
D = 1024          # model dim
DHC = 256         # head dims per core (4 heads x 64)
DH = 64
NQ = 4            # q chunks of 512
NK = 16           # k chunks of 128
NO = 8            # d_in / d_out chunks of 128

_NC_CACHE = None


def _build_core_kernel():
    nc = bacc.Bacc("TRN2", target_bir_lowering=False, debug=False,
                   num_devices=N_CORES)
    xT = nc.dram_tensor("xT", [D, S], BF, kind="ExternalInput").ap()
    w_all = nc.dram_tensor("w_all", [D, 3 * DHC], BF, kind="ExternalInput").ap()
    wo = nc.dram_tensor("wo", [DHC, D], BF, kind="ExternalInput").ap()
    masks = nc.dram_tensor("masks", [128, 128], BF, kind="ExternalInput").ap()
    outT = nc.dram_tensor("outT", [D, S], BF, kind="ExternalOutput").ap()

    with tile.TileContext(nc) as tc:
        _emit(tc, xT, w_all, wo, masks, outT)
    nc.compile()
    return nc


def _emit(tc, xT, w_all, wo, masks, outT):
    nc = tc.nc
    EXPF = mybir.ActivationFunctionType.Exp

    from contextlib import ExitStack
    ctx = ExitStack()
    const = ctx.enter_context(tc.tile_pool(name="const", bufs=1))
    work = ctx.enter_context(tc.tile_pool(name="work", bufs=3))
    recp = ctx.enter_context(tc.tile_pool(name="recp", bufs=2))
    outp = ctx.enter_context(tc.tile_pool(name="outp", bufs=3))
    ps_s = ctx.enter_context(tc.tile_pool(name="ps_s", bufs=2, space="PSUM"))
    ps_c = ctx.enter_context(tc.tile_pool(name="ps_c", bufs=2, space="PSUM"))
    ps_x = ctx.enter_context(tc.tile_pool(name="ps_x", bufs=2, space="PSUM"))

    # ---- persistent SBUF tensors ----
    xt = const.tile([128, NO, S], BF, tag="xt")          # x^T, d_in chunks
    wa = const.tile([128, NO, 3 * DHC], BF, tag="wa")    # [Wq|Wk|Wv] slices
    wos = const.tile([128, 2, D], BF, tag="wos")         # Wo row chunks
    msk = const.tile([128, 128], BF, tag="msk")          # causal staircase
    qt = const.tile([128, 2, S], BF, tag="qt")           # q^T per pair
    kt = const.tile([128, 2, S], BF, tag="kt")           # k^T per pair
    # v + folded ones columns (see module docstring)
    vsb = const.tile([128, NK, 4, 128], BF, tag="vsb")
    ctxT = const.tile([128, 2, S], BF, tag="ctxT")
    ones = const.tile([128, DH], BF, tag="ones")

    nc.sync.dma_start(wa[:], w_all.rearrange("(o p) f -> p o f", p=128))
    nc.sync.dma_start(msk[:], masks)
    nc.sync.dma_start(wos[:], wo.rearrange("(c p) f -> p c f", p=128))
    xTo = xT.rearrange("(o p) s -> o p s", p=128)
    for o in range(NO):  # per-chunk DMAs so matmuls start with chunk 0
        nc.sync.dma_start(xt[:, o, :], xTo[o])
    nc.vector.memset(ones[:], 1.0)
    # odd-head slots: zero cols 0:64, ones col 32 (denominator row source);
    # even-head slots: ones col 64. Cols 65:128 of even slots stay garbage
    # (never read: even lhsT slice is [:, 0:65]).
    nc.vector.memset(vsb[:, :, 1::2, 0:DH], 0.0)
    nc.vector.memset(vsb[:, :, 0::2, DH], 1.0)
    nc.vector.memset(vsb[:, :, 1::2, 32], 1.0)

    # ---- QKV projections ----
    def emit_qk(m, streaming=False):
        # qkvT chunk m: [128 dims, S] = w_all[:, m-slice].T @ x^T
        # streaming=True: o-outer loop so work starts as x^T chunks land.
        if streaming:
            pq0 = ps_s.tile([128, 1024], F32, tag="ps")
            pq1 = ps_s.tile([128, 1024], F32, tag="ps")
            pqs = [pq0, pq1]
            for o in range(NO):
                for n in range(NQ):
                    nc.tensor.matmul(
                        pqs[n // 2][:, 512 * (n % 2):512 * (n % 2) + 512],
                        lhsT=wa[:, o, 128 * m:128 * m + 128],
                        rhs=xt[:, o, 512 * n:512 * n + 512],
                        start=(o == 0), stop=(o == NO - 1),
                        skip_group_check=True)
        for n in range(NQ):
            n_sl = slice(512 * n, 512 * n + 512)
            if streaming:
                pq = pqs[n // 2][:, 512 * (n % 2):512 * (n % 2) + 512]
            else:
                pq = ps_x.tile([128, 512], F32, tag="px")
                for o in range(NO):
                    nc.tensor.matmul(
                        pq[:], lhsT=wa[:, o, 128 * m:128 * m + 128],
                        rhs=xt[:, o, n_sl],
                        start=(o == 0), stop=(o == NO - 1))
            if m < 2:
                nc.vector.tensor_copy(qt[:, m, n_sl], pq[:])
            else:
                nc.vector.tensor_copy(kt[:, m - 2, n_sl], pq[:])

    def emit_v():
        # v [tokens, 4*dh] = x @ Wv  (x^T chunks are the stationary side)
        for t in range(NK):
            pv = ps_x.tile([128, 512], F32, tag="px")
            for o in range(NO):
                nc.tensor.matmul(
                    pv[:, :DHC], lhsT=xt[:, o, 128 * t:128 * t + 128],
                    rhs=wa[:, o, 2 * DHC:3 * DHC],
                    start=(o == 0), stop=(o == NO - 1))
            pv4 = pv[:, :DHC].rearrange("p (h c) -> p h c", c=DH)
            # even heads -> cols 0:64, odd heads -> cols 64:128
            nc.vector.tensor_copy(vsb[:, t, 0::2, 0:DH], pv4[:, 0::2, :])
            nc.vector.tensor_copy(vsb[:, t, 1::2, DH:128], pv4[:, 1::2, :])

    # ---- attention for one (pair, q-window) ----
    def emit_attn_chunk(p, j):
        n_i = 4 * j + 4
        q_sl = slice(512 * j, 512 * j + 512)
        pcA = ps_c.tile([128, 512], F32, tag="pc")
        pcB = ps_c.tile([128, 512], F32, tag="pc")
        for i in range(n_i):
            k_sl = slice(128 * i, 128 * i + 128)
            d = i - 4 * j
            # diagonal tiles: k-chunk i only reaches q >= 128*d in this
            # q-window; restrict all work to the valid column range.
            q0 = 128 * d if d > 0 else 0
            qv_sl = slice(512 * j + q0, 512 * j + 512)
            pss = ps_s.tile([128, 1024], F32, tag="ps")
            # two heads as concurrent row-tiled K=64 matmuls
            nc.tensor.matmul(pss[:, q0:512],
                             lhsT=kt[0:64, p, k_sl], rhs=qt[0:64, p, qv_sl],
                             start=True, stop=True)
            nc.tensor.matmul(pss[:, 512 + q0:1024],
                             lhsT=kt[64:128, p, k_sl],
                             rhs=qt[64:128, p, qv_sl],
                             start=True, stop=True)
            eT = work.tile([128, 2, 512], BF, tag="exp")
            pv2 = pss.rearrange("p (g f) -> p g f", g=2)
            nc.scalar.activation(eT[:, :, q0:512], pv2[:, :, q0:512],
                                 EXPF, scale=0.125)
            if d >= 0:  # triangular 128x128 mask block on the diagonal
                for h in (0, 1):
                    nc.vector.tensor_mul(eT[:, h, q0:q0 + 128],
                                         eT[:, h, q0:q0 + 128], msk[:])
            # ctx accumulation; ones columns accumulate denominators
            nc.tensor.matmul(
                pcA[0:65, q0:512], lhsT=vsb[:, i, 2 * p, 0:65],
                rhs=eT[:, 0, q0:512],
                start=(i == 0), stop=(i == n_i - 1), skip_group_check=True)
            nc.tensor.matmul(
                pcB[:, q0:512], lhsT=vsb[:, i, 2 * p + 1, :],
                rhs=eT[:, 1, q0:512],
                start=(i == 0), stop=(i == n_i - 1), skip_group_check=True)
        # ---- window tail: reciprocal, raw evacuation, normalize ----
        # stage denominators to SBUF (bf16), replicate across the head's 64
        # partitions with K=1 ones-matmuls, then one base-0 reciprocal.
        # (reciprocal_approx_fast/partition_broadcast silently misbehave on
        # HW at base partition != 0, so the recip must run from partition 0.)
        den = recp.tile([128, 512], BF, tag="den")
        rec = recp.tile([128, 512], F32, tag="rec")
        nc.vector.tensor_copy(den[64:65, :], pcA[64:65, :])
        nc.vector.tensor_copy(ctxT[0:64, p, q_sl], pcA[0:64, :])
        nc.vector.tensor_copy(den[32:33, :], pcB[32:33, :])
        nc.vector.tensor_copy(ctxT[64:128, p, q_sl], pcB[64:128, :])
        pd = ps_x.tile([128, 512], F32, tag="px")
        nc.tensor.matmul(pd[0:64, :], lhsT=ones[64:65, :],
                         rhs=den[64:65, :],
                         start=True, stop=True, tile_position=(64, 0))
        nc.tensor.matmul(pd[64:128, :], lhsT=ones[32:33, :],
                         rhs=den[32:33, :],
                         start=True, stop=True, tile_position=(32, 64))
        nc.vector.reciprocal_approx_fast(out=rec[:, :], in_=pd[:, :])
        nc.vector.tensor_mul(ctxT[0:64, p, q_sl], ctxT[0:64, p, q_sl],
                             rec[0:64, :])
        nc.vector.tensor_mul(ctxT[64:128, p, q_sl], ctxT[64:128, p, q_sl],
                             rec[64:128, :])

    # ---- out projection for one token block: outT[:, n] += wo.T @ ctxT ----
    outT_m = outT.rearrange("(mm p) s -> mm p s", p=128)

    def emit_outproj_n(n):
        n_sl = slice(512 * n, 512 * n + 512)
        for m in range(NO):
            po = ps_x.tile([128, 512], F32, tag="px")
            for p in (0, 1):
                nc.tensor.matmul(
                    po[:], lhsT=wos[:, p, 128 * m:128 * m + 128],
                    rhs=ctxT[:, p, n_sl],
                    start=(p == 0), stop=(p == 1))
            osb = outp.tile([128, 512], BF, tag="osb")
            nc.vector.tensor_copy(osb[:], po[:])
            nc.sync.dma_start(outT_m[m, :, n_sl], osb[:])

    emit_qk(0, streaming=True)
    emit_qk(2, streaming=True)
    emit_v()
    emit_attn_chunk(0, 3)
    emit_qk(1)
    emit_qk(3)
    for j in (3, 2, 1, 0):
        if j != 3:
            emit_attn_chunk(0, j)
        emit_attn_chunk(1, j)
        emit_outproj_n(j)
    ctx.close()


def _get_nc():
    global _NC_CACHE
    if _NC_CACHE is None:
        _NC_CACHE = _build_core_kernel()
    return _NC_CACHE


def _build_masks():
    p = np.arange(128)[:, None]
    f = np.arange(128)[None, :]
    return (p <= f).astype(BF16)


def _shard_inputs(x, Wq, Wk, Wv, Wo):
    xb = x.astype(BF16)
    masks = _build_masks()
    in_maps = []
    for c in range(N_CORES):
        b, g = divmod(c, 4)
        cols = slice(DHC * g, DHC * g + DHC)
        w_all = np.ascontiguousarray(np.concatenate(
            [Wq[:, cols], Wk[:, cols], Wv[:, cols]], axis=1).astype(BF16))
        wo_s = np.ascontiguousarray(Wo[cols, :].astype(BF16))
        xT = np.ascontiguousarray(xb[b].T)
        in_maps.append({"xT": xT, "w_all": w_all, "wo": wo_s, "masks": masks})
    return in_maps


def _unshard(results, bo):
    out = np.empty((2, S, D), np.float32)
    for b in range(2):
        acc = results[4 * b]["outT"].astype(np.float32)
        for g in range(1, 4):
            acc += results[4 * b + g]["outT"].astype(np.float32)
        out[b] = acc.T + bo.astype(np.float32)
    return out


def run(x, Wq, Wk, Wv, Wo, bo, trace=False, **spmd_kwargs):
    nc = _get_nc()
    in_maps = _shard_inputs(x, Wq, Wk, Wv, Wo)
    res = bass_utils.run_bass_kernel_spmd(
        nc, in_maps, core_ids=list(range(N_CORES)), trace=trace,
        **spmd_kwargs)
    return _unshard(res.results, bo), res


def kernel(x, Wq, Wk, Wv, Wo, bo):
    out, _ = run(np.asarray(x), np.asarray(Wq), np.asarray(Wk),
                 np.asarray(Wv), np.asarray(Wo), np.asarray(bo))
    return out


# revision 12
# speedup vs baseline: 1.2181x; 1.0675x over previous
"""Multi-head causal attention on 8 Trainium2 NeuronCores.

Sharding: core c handles batch b=c//4, head group g=c%4 (4 heads of 16).
Per-core Bass kernel computes QKV projection, causal attention in a
transposed-scores layout, and the out-projection partial; the host sums
the 4 per-batch bf16 partials (the out_proj all-reduce) in fp32 + bias.

v2 layout notes (per core, S=2048 tokens, D=1024, 4 heads x dh=64):
  - qt/kt [128, pair, S] bf16: partitions 0:64 = even head dh, 64:128 =
    odd head dh. No zero padding.
  - scores: the two heads of a pair run as CONCURRENT row-tiled K=64
    matmuls (tile_position (0,0) and (64,0)) into two PSUM banks of one
    [128, 1024] tile -> ~2x score throughput vs serial K=128.
  - exp: one ACT per k-chunk over both heads' banks ([128, 2, 512-q0]).
  - ctx: per head one matmul accumulating over k-chunks. A ones-column
    is folded into the V stationary operand so the softmax DENOMINATOR
    accumulates in the same PSUM bank for free (no DVE row-sum chain):
      even head: lhsT = [v(64) | 1] (M=65)  -> ctx at rows 0:64, den row 64
      odd head:  lhsT = [0(32)|1|0(31)|v(64)] (M=128) -> den row 32,
                 ctx at rows 64:128 (lane-aligned with ctxT's B half).
  - denominators: DVE reciprocal_approx_fast on the den rows, then K=1
    ones-matmuls (bf16 x f32r) replicate 1/den across 64 partitions;
    ctxT (staged raw in bf16) is normalized in place by two DVE muls.
  - out^T partial [D, S] bf16 = wo.T @ ctxT, accumulated over the 2
    pairs; host sums partials in fp32.
"""

import sys

sys.path.insert(0, "/opt/trn_rl_repo")

import numpy as np
import ml_dtypes

import concourse.bass as bass
import concourse.tile as tile
from concourse import bacc, mybir
from concourse import bass_utils

BF16 = ml_dtypes.bfloat16
F32 = mybir.dt.float32
F32R = mybir.dt.float32r
BF = mybir.dt.bfloat16

N_CORES = 8
S = 2048          # tokens
D = 1024          # model dim
DHC = 256         # head dims per core (4 heads x 64)
DH = 64
NQ = 4            # q chunks of 512
NK = 16           # k chunks of 128
NO = 8            # d_in / d_out chunks of 128

_NC_CACHE = None


def _build_core_kernel():
    nc = bacc.Bacc("TRN2", target_bir_lowering=False, debug=False,
                   num_devices=N_CORES)
    xT = nc.dram_tensor("xT", [D, S], BF, kind="ExternalInput").ap()
    w_all = nc.dram_tensor("w_all", [D, 3 * DHC], BF, kind="ExternalInput").ap()
    wo = nc.dram_tensor("wo", [DHC, D], BF, kind="ExternalInput").ap()
    masks = nc.dram_tensor("masks", [128, 128], BF, kind="ExternalInput").ap()
    outT = nc.dram_tensor("outT", [D, S], BF, kind="ExternalOutput").ap()

    with tile.TileContext(nc) as tc:
        _emit(tc, xT, w_all, wo, masks, outT)
    nc.compile()
    return nc


def _emit(tc, xT, w_all, wo, masks, outT):
    nc = tc.nc
    EXPF = mybir.ActivationFunctionType.Exp

    from contextlib import ExitStack
    ctx = ExitStack()
    const = ctx.enter_context(tc.tile_pool(name="const", bufs=1))
    work = ctx.enter_context(tc.tile_pool(name="work", bufs=3))
    recp = ctx.enter_context(tc.tile_pool(name="recp", bufs=2))
    outp = ctx.enter_context(tc.tile_pool(name="outp", bufs=3))
    ps_s = ctx.enter_context(tc.tile_pool(name="ps_s", bufs=2, space="PSUM"))
    ps_c = ctx.enter_context(tc.tile_pool(name="ps_c", bufs=2, space="PSUM"))
    ps_x = ctx.enter_context(tc.tile_pool(name="ps_x", bufs=2, space="PSUM"))

    # ---- persistent SBUF tensors ----
    xt = const.tile([128, NO, S], BF, tag="xt")          # x^T, d_in chunks
    wa = const.tile([128, NO, 3 * DHC], BF, tag="wa")    # [Wq|Wk|Wv] slices
    wos = const.tile([128, 2, D], BF, tag="wos")         # Wo row chunks
    msk = const.tile([128, 128], BF, tag="msk")          # causal staircase
    qt = const.tile([128, 2, S], BF, tag="qt")           # q^T per pair
    kt = const.tile([128, 2, S], BF, tag="kt")           # k^T per pair
    # v + folded ones columns (see module docstring)
    vsb = const.tile([128, NK, 4, 128], BF, tag="vsb")
    ctxT = const.tile([128, 2, S], BF, tag="ctxT")
    ones = const.tile([128, DH], BF, tag="ones")

    nc.sync.dma_start(wa[:], w_all.rearrange("(o p) f -> p o f", p=128))
    nc.sync.dma_start(msk[:], masks)
    nc.sync.dma_start(wos[:], wo.rearrange("(c p) f -> p c f", p=128))
    xTo = xT.rearrange("(o p) s -> o p s", p=128)
    for o in range(NO):  # per-chunk DMAs so matmuls start with chunk 0
        nc.sync.dma_start(xt[:, o, :], xTo[o])
    nc.vector.memset(ones[:], 1.0)
    # odd-head slots: zero cols 0:64, ones col 32 (denominator row source);
    # even-head slots: ones col 64. Cols 65:128 of even slots stay garbage
    # (never read: even lhsT slice is [:, 0:65]).
    nc.vector.memset(vsb[:, :, 1::2, 0:DH], 0.0)
    nc.vector.memset(vsb[:, :, 0::2, DH], 1.0)
    nc.vector.memset(vsb[:, :, 1::2, 32], 1.0)

    # ---- QKV projections ----
    def emit_qk(m, streaming=False):
        # qkvT chunk m: [128 dims, S] = w_all[:, m-slice].T @ x^T
        # streaming=True: o-outer loop so work starts as x^T chunks land.
        if streaming:
            pq0 = ps_s.tile([128, 1024], F32, tag="ps")
            pq1 = ps_s.tile([128, 1024], F32, tag="ps")
            pqs = [pq0, pq1]
            for o in range(NO):
                for n in range(NQ):
                    nc.tensor.matmul(
                        pqs[n // 2][:, 512 * (n % 2):512 * (n % 2) + 512],
                        lhsT=wa[:, o, 128 * m:128 * m + 128],
                        rhs=xt[:, o, 512 * n:512 * n + 512],
                        start=(o == 0), stop=(o == NO - 1),
                        skip_group_check=True)
        for n in range(NQ):
            n_sl = slice(512 * n, 512 * n + 512)
            if streaming:
                pq = pqs[n // 2][:, 512 * (n % 2):512 * (n % 2) + 512]
            else:
                pq = ps_x.tile([128, 512], F32, tag="px")
                for o in range(NO):
                    nc.tensor.matmul(
                        pq[:], lhsT=wa[:, o, 128 * m:128 * m + 128],
                        rhs=xt[:, o, n_sl],
                        start=(o == 0), stop=(o == NO - 1))
            if m < 2:
                nc.vector.tensor_copy(qt[:, m, n_sl], pq[:])
            else:
                nc.vector.tensor_copy(kt[:, m - 2, n_sl], pq[:])

    def emit_v():
        # v [tokens, 4*dh] = x @ Wv  (x^T chunks are the stationary side)
        for t in range(NK):
            pv = ps_x.tile([128, 512], F32, tag="px")
            for o in range(NO):
                nc.tensor.matmul(
                    pv[:, :DHC], lhsT=xt[:, o, 128 * t:128 * t + 128],
                    rhs=wa[:, o, 2 * DHC:3 * DHC],
                    start=(o == 0), stop=(o == NO - 1))
            pv4 = pv[:, :DHC].rearrange("p (h c) -> p h c", c=DH)
            # even heads -> cols 0:64, odd heads -> cols 64:128
            nc.vector.tensor_copy(vsb[:, t, 0::2, 0:DH], pv4[:, 0::2, :])
            nc.vector.tensor_copy(vsb[:, t, 1::2, DH:128], pv4[:, 1::2, :])

    # ---- attention for one (pair, q-window) ----
    def make_qk_filler(ms):
        # generator of single-instruction steps computing qkvT chunks for
        # the given m-chunks; spliced into attention windows as PE filler.
        for m in ms:
            for n in range(NQ):
                n_sl = slice(512 * n, 512 * n + 512)
                pq = ps_x.tile([128, 512], F32, tag="px")
                for o in range(NO):
                    nc.tensor.matmul(
                        pq[:], lhsT=wa[:, o, 128 * m:128 * m + 128],
                        rhs=xt[:, o, n_sl],
                        start=(o == 0), stop=(o == NO - 1),
                        skip_group_check=True)
                    yield
                if m < 2:
                    nc.vector.tensor_copy(qt[:, m, n_sl], pq[:])
                else:
                    nc.vector.tensor_copy(kt[:, m - 2, n_sl], pq[:])
                yield

    def emit_attn_chunk(p, j, filler=None, per_i=0, pending_tail=None):
        n_i = 4 * j + 4
        q_sl = slice(512 * j, 512 * j + 512)
        pcA = ps_c.tile([128, 512], F32, tag="pc")
        pcB = ps_c.tile([128, 512], F32, tag="pc")

        def emit_scores(i):
            d = i - 4 * j
            # diagonal tiles: k-chunk i only reaches q >= 128*d in this
            # q-window; restrict all work to the valid column range.
            q0 = 128 * d if d > 0 else 0
            k_sl = slice(128 * i, 128 * i + 128)
            qv_sl = slice(512 * j + q0, 512 * j + 512)
            pss = ps_s.tile([128, 1024], F32, tag="ps")
            # two heads as concurrent row-tiled K=64 matmuls
            nc.tensor.matmul(pss[:, q0:512],
                             lhsT=kt[0:64, p, k_sl], rhs=qt[0:64, p, qv_sl],
                             start=True, stop=True)
            nc.tensor.matmul(pss[:, 512 + q0:1024],
                             lhsT=kt[64:128, p, k_sl],
                             rhs=qt[64:128, p, qv_sl],
                             start=True, stop=True)
            return pss, q0

        # software pipeline: scores for i+1 are emitted before ctx of i so
        # the PE never sits behind a wait on the exp of i.
        pss_cur, q0_cur = emit_scores(0)
        if pending_tail is not None:
            pending_tail()
        for i in range(n_i):
            q0 = q0_cur
            eT = work.tile([128, 2, 512], BF, tag="exp")
            pv2 = pss_cur.rearrange("p (g f) -> p g f", g=2)
            nc.scalar.activation(eT[:, :, q0:512], pv2[:, :, q0:512],
                                 EXPF, scale=0.125)
            if i + 1 < n_i:
                pss_cur, q0_cur = emit_scores(i + 1)
            if filler is not None:
                for _ in range(per_i):
                    next(filler, None)
            if i - 4 * j >= 0:  # triangular 128x128 mask on the diagonal
                for h in (0, 1):
                    nc.vector.tensor_mul(eT[:, h, q0:q0 + 128],
                                         eT[:, h, q0:q0 + 128], msk[:])
            # ctx accumulation; ones columns accumulate denominators
            nc.tensor.matmul(
                pcA[0:65, q0:512], lhsT=vsb[:, i, 2 * p, 0:65],
                rhs=eT[:, 0, q0:512],
                start=(i == 0), stop=(i == n_i - 1), skip_group_check=True)
            nc.tensor.matmul(
                pcB[:, q0:512], lhsT=vsb[:, i, 2 * p + 1, :],
                rhs=eT[:, 1, q0:512],
                start=(i == 0), stop=(i == n_i - 1), skip_group_check=True)
        # ---- window tail: reciprocal, raw evacuation, normalize ----
        # window tail, returned as a closure so the caller can emit it AFTER
        # the next window's first scores (keeps the PE stream dense):
        # stage denominators to SBUF (bf16), replicate across the head's 64
        # partitions with K=1 ones-matmuls, then one base-0 reciprocal.
        # (reciprocal_approx_fast/partition_broadcast silently misbehave on
        # HW at base partition != 0, so the recip must run from partition 0.)
        def tail():
            den = recp.tile([128, 512], BF, tag="den")
            rec = recp.tile([128, 512], F32, tag="rec")
            nc.vector.tensor_copy(den[64:65, :], pcA[64:65, :])
            nc.vector.tensor_copy(ctxT[0:64, p, q_sl], pcA[0:64, :])
            nc.vector.tensor_copy(den[32:33, :], pcB[32:33, :])
            nc.vector.tensor_copy(ctxT[64:128, p, q_sl], pcB[64:128, :])
            pd = ps_x.tile([128, 512], F32, tag="px")
            nc.tensor.matmul(pd[0:64, :], lhsT=ones[64:65, :],
                             rhs=den[64:65, :],
                             start=True, stop=True, tile_position=(64, 0))
            nc.tensor.matmul(pd[64:128, :], lhsT=ones[32:33, :],
                             rhs=den[32:33, :],
                             start=True, stop=True, tile_position=(32, 64))
            nc.vector.reciprocal_approx_fast(out=rec[:, :], in_=pd[:, :])
            nc.vector.tensor_mul(ctxT[0:64, p, q_sl], ctxT[0:64, p, q_sl],
                                 rec[0:64, :])
            nc.vector.tensor_mul(ctxT[64:128, p, q_sl],
                                 ctxT[64:128, p, q_sl], rec[64:128, :])
        return tail

    # ---- out projection for one token block: outT[:, n] += wo.T @ ctxT ----
    outT_m = outT.rearrange("(mm p) s -> mm p s", p=128)

    def emit_outproj_n(n):
        n_sl = slice(512 * n, 512 * n + 512)
        for m in range(NO):
            po = ps_x.tile([128, 512], F32, tag="px")
            for p in (0, 1):
                nc.tensor.matmul(
                    po[:], lhsT=wos[:, p, 128 * m:128 * m + 128],
                    rhs=ctxT[:, p, n_sl],
                    start=(p == 0), stop=(p == 1))
            osb = outp.tile([128, 512], BF, tag="osb")
            nc.vector.tensor_copy(osb[:], po[:])
            nc.sync.dma_start(outT_m[m, :, n_sl], osb[:])

    emit_qk(0, streaming=True)
    emit_qk(2, streaming=True)
    emit_v()
    # pair-1 QKV projections are spliced into the first two (pair-0)
    # windows as PE filler under their ACT-bound stretches.
    fill = make_qk_filler([1, 3])
    t = emit_attn_chunk(0, 3, filler=fill, per_i=3)
    t = emit_attn_chunk(0, 2, filler=fill, per_i=3, pending_tail=t)
    for _ in fill:  # drain any remaining filler steps
        pass
    t = emit_attn_chunk(1, 3, pending_tail=t)
    t = emit_attn_chunk(1, 2, pending_tail=t)   # emits (1,3)'s tail
    emit_outproj_n(3)
    t = emit_attn_chunk(0, 1, pending_tail=t)   # emits (1,2)'s tail
    emit_outproj_n(2)
    t = emit_attn_chunk(0, 0, pending_tail=t)
    t = emit_attn_chunk(1, 1, pending_tail=t)
    t = emit_attn_chunk(1, 0, pending_tail=t)   # emits (1,1)'s tail
    emit_outproj_n(1)
    t()
    emit_outproj_n(0)
    ctx.close()


def _get_nc():
    global _NC_CACHE
    if _NC_CACHE is None:
        _NC_CACHE = _build_core_kernel()
    return _NC_CACHE


def _build_masks():
    p = np.arange(128)[:, None]
    f = np.arange(128)[None, :]
    return (p <= f).astype(BF16)


def _shard_inputs(x, Wq, Wk, Wv, Wo):
    xb = x.astype(BF16)
    masks = _build_masks()
    in_maps = []
    for c in range(N_CORES):
        b, g = divmod(c, 4)
        cols = slice(DHC * g, DHC * g + DHC)
        w_all = np.ascontiguousarray(np.concatenate(
            [Wq[:, cols], Wk[:, cols], Wv[:, cols]], axis=1).astype(BF16))
        wo_s = np.ascontiguousarray(Wo[cols, :].astype(BF16))
        xT = np.ascontiguousarray(xb[b].T)
        in_maps.append({"xT": xT, "w_all": w_all, "wo": wo_s, "masks": masks})
    return in_maps


def _unshard(results, bo):
    out = np.empty((2, S, D), np.float32)
    for b in range(2):
        acc = results[4 * b]["outT"].astype(np.float32)
        for g in range(1, 4):
            acc += results[4 * b + g]["outT"].astype(np.float32)
        out[b] = acc.T + bo.astype(np.float32)
    return out


def run(x, Wq, Wk, Wv, Wo, bo, trace=False, **spmd_kwargs):
    nc = _get_nc()
    in_maps = _shard_inputs(x, Wq, Wk, Wv, Wo)
    res = bass_utils.run_bass_kernel_spmd(
        nc, in_maps, core_ids=list(range(N_CORES)), trace=trace,
        **spmd_kwargs)
    return _unshard(res.results, bo), res


def kernel(x, Wq, Wk, Wv, Wo, bo):
    out, _ = run(np.asarray(x), np.asarray(Wq), np.asarray(Wk),
                 np.asarray(Wv), np.asarray(Wo), np.asarray(bo))
    return out
